# revision 2
# baseline (speedup 1.0000x reference)
"""Trainium2 Bass kernel for nn_Attention_50740743635025.

Math (fused form of the reference):
  s[c]          = dot(w_key[c], w_query[c])                    # [C]
  S[c, f]       = sum_b y[b, c] * x[b, f]                      # [C, F] (full batch)
  scores[c, f]  = s[c] / B * S[c, f]
  attn[c, f]    = exp(scores[c, f] - max_f scores[c, :])       # softmax/max cancels
  Weff[c, f, o] = attn[c, f] * w_value[c] * weight[c, f, o]
  out[c, b, o]  = sum_f x[b, f] * Weff[c, f, o] + bias[c, o]

Sharding: data-parallel over batch B across 8 cores (1024 rows each).
Each core computes its partial S over its batch shard; an AllGather of
the [C, F] partials (10 KB) plus a local 8-way sum reproduces the full
batch reduction on every core.  Each core then computes attn (identical
on all cores) and its own [1024, C*O] slice of the output.

Output layout per core is [B_shard, C*O] (batch-major) so HBM writes
are fully contiguous; the host gather transposes back to [C, B, O].
"""

import numpy as np

import concourse.bass as bass
import concourse.mybir as mybir
import concourse.tile as tile
from concourse import bacc
from concourse import bass_utils
from concourse.masks import make_identity

C, B, F, O, DK = 10, 8192, 256, 64, 5
NCORES = 8
BS = B // NCORES            # 1024 batch rows per core
NT = BS // 128              # 8 batch tiles per core
FT = F // 128               # 2 feature tiles
CO = C * O                  # 640
NH = 2                      # output free-dim split (2 x 320 <= 512 fp32 limit)
CO_H = CO // NH             # 320
F32 = mybir.dt.float32
AF = mybir.ActivationFunctionType
AX = mybir.AxisListType

_CACHE = {}
LAST_RESULTS = None


def _emit(nc, tc, aps):
    x_d, y_d, wk_d, wq_d, wv_d, w_d, b_d, out_d = aps
    with (
        tc.tile_pool(name="consts", bufs=1) as consts,
        tc.tile_pool(name="xpool", bufs=1) as xpool,
        tc.tile_pool(name="stage", bufs=3) as stage_pool,
        tc.tile_pool(name="psum_big", bufs=2, space="PSUM") as psum_big,
        tc.tile_pool(name="psum_small", bufs=2, space="PSUM") as psum_small,
        tc.tile_pool(name="dram", bufs=1, space="DRAM") as dram,
    ):
        # ---- constants / parameter loads --------------------------------
        identity = consts.tile([128, 128], F32)
        make_identity(nc, identity)
        ones_row = consts.tile([1, 128], F32)
        nc.gpsimd.memset(ones_row, 1.0)

        wk_sb = consts.tile([C, DK], F32)
        wq_sb = consts.tile([C, DK], F32)
        wv_sb = consts.tile([C, 1], F32)
        bias_sb = consts.tile([1, CO], F32)
        nc.sync.dma_start(wk_sb, wk_d)
        nc.sync.dma_start(wq_sb, wq_d)
        nc.sync.dma_start(wv_sb, wv_d)
        nc.sync.dma_start(bias_sb, b_d)

        w_sb = []
        for h in range(FT):
            wt = consts.tile([128, CO], F32, name=f"w_sb{h}")
            nc.sync.dma_start(
                wt.rearrange("p (c o) -> p c o", c=C),
                w_d[:, h * 128:(h + 1) * 128, :].rearrange("c f o -> f c o"),
            )
            w_sb.append(wt)

        # ---- x / y shard loads (x in chunks so matmul starts early) -----
        x_sb = xpool.tile([128, NT, F], F32)
        xr = x_d.rearrange("(t p) f -> p t f", p=128)
        XCH = 2  # tiles per input DMA chunk
        for i in range(0, NT, XCH):
            nc.sync.dma_start(x_sb[:, i:i + XCH], xr[:, i:i + XCH])
        y_sb = xpool.tile([128, NT, C], F32)
        nc.sync.dma_start(y_sb, y_d.rearrange("(t p) c -> p t c", p=128))

        # ---- partial scores S_part[c, f] = sum_{b in shard} y[b,c]x[b,f]
        ps_scores = psum_small.tile([C, F], F32, bufs=1)
        for t in range(NT):
            nc.tensor.matmul(ps_scores, y_sb[:, t], x_sb[:, t],
                             start=(t == 0), stop=(t == NT - 1))
        sp_sb = consts.tile([C, F], F32)
        nc.vector.tensor_copy(sp_sb, ps_scores)

        # ---- AllGather the partials (runs on TOPSP/SDMA silicon) --------
        cc_in = dram.tile([C, F], F32)
        cc_out = dram.tile([NCORES * C, F], F32, addr_space="Shared")
        nc.sync.dma_start(cc_in, sp_sb)
        nc.gpsimd.collective_compute(
            "AllGather",
            mybir.AluOpType.bypass,
            replica_groups=[list(range(NCORES))],
            ins=[cc_in.opt()],
            outs=[cc_out.opt()],
        )

        # ---- work independent of the collective: transposes, bias -------
        # x^T tiles [128 f, BS b] via PE transpose (hidden under AllGather)
        xT = [xpool.tile([128, BS], F32, name=f"xT{h}") for h in range(FT)]
        for t in range(NT):
            for h in range(FT):
                ps_tr = psum_small.tile([128, 128], F32, tag="ps_tr")
                nc.tensor.transpose(ps_tr, x_sb[:, t, h * 128:(h + 1) * 128],
                                    identity)
                nc.vector.tensor_copy(xT[h][:, t * 128:(t + 1) * 128], ps_tr)

        # bias broadcast [128, CO] via ones-column matmul
        bias_bc = consts.tile([128, CO], F32)
        for nh in range(NH):
            ps_b = psum_big.tile([128, CO_H], F32, tag=f"ps_o{nh}")
            nc.tensor.matmul(ps_b, ones_row,
                             bias_sb[:, nh * CO_H:(nh + 1) * CO_H],
                             start=True, stop=True)
            nc.vector.tensor_copy(bias_bc[:, nh * CO_H:(nh + 1) * CO_H], ps_b)

        # ---- gather + sum -> full S -------------------------------------
        gath = consts.tile([C, NCORES, F], F32)
        nc.sync.dma_start(gath, cc_out.rearrange("(r c) f -> c r f", c=C))
        ssum = consts.tile([C, F], F32)
        nc.vector.tensor_add(ssum, gath[:, 0], gath[:, 1])
        for r in range(2, NCORES):
            nc.vector.tensor_add(ssum, ssum, gath[:, r])

        # ---- attn = exp(scores - rowmax) * w_value ----------------------
        kq = consts.tile([C, DK], F32)
        nc.vector.tensor_mul(kq, wk_sb, wq_sb)
        skq = consts.tile([C, 1], F32)
        nc.vector.reduce_sum(skq, kq, axis=AX.X)
        nc.scalar.mul(skq, skq, 1.0 / B)
        scores = consts.tile([C, F], F32)
        nc.vector.tensor_scalar_mul(scores, ssum, skq)
        rowmax = consts.tile([C, 1], F32)
        nc.vector.reduce_max(rowmax, scores, axis=AX.X)
        neg_max = consts.tile([C, 1], F32)
        nc.scalar.mul(neg_max, rowmax, -1.0)
        attn = consts.tile([C, F], F32)
        nc.scalar.activation(attn, scores, AF.Exp, bias=neg_max, scale=1.0)
        nc.vector.tensor_scalar_mul(attn, attn, wv_sb)

        # ---- attn^T as [128 f, C] tiles ---------------------------------
        attnT = consts.tile([128, FT, C], F32)
        for h in range(FT):
            ps_at = psum_small.tile([128, C], F32, tag="ps_tr")
            nc.tensor.transpose(ps_at, attn[:, h * 128:(h + 1) * 128],
                                identity[:C, :C])
            nc.vector.tensor_copy(attnT[:, h], ps_at)

        # ---- Weff[h] = w_sb[h] * attnT[:, h, c] (broadcast over o) ------
        weff = []
        for h in range(FT):
            wf = consts.tile([128, CO], F32, name=f"weff{h}")
            for c in range(C):
                sl = slice(c * O, (c + 1) * O)
                if c % 2 == 0:
                    nc.vector.tensor_scalar_mul(wf[:, sl], w_sb[h][:, sl],
                                                attnT[:, h, c:c + 1])
                else:
                    nc.scalar.activation(wf[:, sl], w_sb[h][:, sl], AF.Copy,
                                         scale=attnT[:, h, c:c + 1])
            weff.append(wf)

        # ---- big matmul: out[b, (c o)] = x @ Weff + bias ----------------
        for t in range(NT):
            ps_o = [psum_big.tile([128, CO_H], F32, tag=f"ps_o{nh}",
                                  name=f"ps_o{nh}_{t}")
                    for nh in range(NH)]
            for h in range(FT):
                lhsT = xT[h][:, t * 128:(t + 1) * 128]
                for nh in range(NH):
                    nc.tensor.matmul(
                        ps_o[nh], lhsT,
                        weff[h][:, nh * CO_H:(nh + 1) * CO_H],
                        start=(h == 0), stop=(h == FT - 1))
            st = stage_pool.tile([128, CO], F32, tag="st")
            for nh in range(NH):
                sl = slice(nh * CO_H, (nh + 1) * CO_H)
                nc.vector.tensor_add(st[:, sl], ps_o[nh], bias_bc[:, sl])
            nc.scalar.dma_start(out_d[t * 128:(t + 1) * 128, :], st)


def _build():
    nc = bacc.Bacc("TRN2", target_bir_lowering=False, debug=False,
                   enable_asserts=False, num_devices=NCORES)
    x_d = nc.dram_tensor("x_s", [BS, F], F32, kind="ExternalInput").ap()
    y_d = nc.dram_tensor("y_s", [BS, C], F32, kind="ExternalInput").ap()
    wk_d = nc.dram_tensor("w_key", [C, DK], F32, kind="ExternalInput").ap()
    wq_d = nc.dram_tensor("w_query", [C, DK], F32, kind="ExternalInput").ap()
    wv_d = nc.dram_tensor("w_value", [C, 1], F32, kind="ExternalInput").ap()
    w_d = nc.dram_tensor("weight", [C, F, O], F32, kind="ExternalInput").ap()
    b_d = nc.dram_tensor("bias", [1, CO], F32, kind="ExternalInput").ap()
    out_d = nc.dram_tensor("out", [BS, CO], F32, kind="ExternalOutput").ap()

    with tile.TileContext(nc) as tc:
        _emit(nc, tc, (x_d, y_d, wk_d, wq_d, wv_d, w_d, b_d, out_d))
    nc.compile()
    return nc


def get_nc():
    if "nc" not in _CACHE:
        _CACHE["nc"] = _build()
    return _CACHE["nc"]


def make_in_maps(x, y, w_key, w_query, w_value, weight, bias):
    f = np.float32
    wk2 = np.ascontiguousarray(w_key.reshape(C, DK), dtype=f)
    wq2 = np.ascontiguousarray(w_query.reshape(C, DK), dtype=f)
    wv2 = np.ascontiguousarray(w_value.reshape(C, 1), dtype=f)
    w3 = np.ascontiguousarray(weight.reshape(C, F, O), dtype=f)
    b2 = np.ascontiguousarray(bias.reshape(1, CO), dtype=f)
    in_maps = []
    for r in range(NCORES):
        in_maps.append({
            "x_s": np.ascontiguousarray(x[r * BS:(r + 1) * BS], dtype=f),
            "y_s": np.ascontiguousarray(y[r * BS:(r + 1) * BS], dtype=f),
            "w_key": wk2,
            "w_query": wq2,
            "w_value": wv2,
            "weight": w3,
            "bias": b2,
        })
    return in_maps


def kernel(x, y, w_key, w_query, w_value, weight, bias):
    global LAST_RESULTS
    nc = get_nc()
    in_maps = make_in_maps(x, y, w_key, w_query, w_value, weight, bias)
    res = bass_utils.run_bass_kernel_spmd(nc, in_maps,
                                          core_ids=list(range(NCORES)))
    LAST_RESULTS = res
    # each core: [BS, C*O] -> full out [C, B, O]
    shards = [np.asarray(res.results[r]["out"]).reshape(BS, C, O)
              for r in range(NCORES)]
    full = np.concatenate(shards, axis=0)          # [B, C, O]
    return np.ascontiguousarray(full.transpose(1, 0, 2))


# revision 14
# speedup vs baseline: 2.6790x; 2.6790x over previous
"""Trainium2 Bass kernel for nn_Attention_50740743635025.

Math (fused form of the reference):
  s[c]          = dot(w_key[c], w_query[c])                    # [C]
  S[c, f]       = sum_b y[b, c] * x[b, f]                      # [C, F] (full batch)
  scores[c, f]  = s[c] / B * S[c, f]
  attn[c, f]    = exp(scores[c, f] - max_f scores[c, :])       # softmax/max cancels
  Weff[c, f, o] = attn[c, f] * w_value[c] * weight[c, f, o]
  out[c, b, o]  = sum_f x[b, f] * Weff[c, f, o] + bias[c, o]

Sharding: the output is data-parallel over batch B across 8 cores (1024
rows each).  The tiny [C, F] scores reduction needs the FULL batch, so
every core redundantly streams all of x (8.4 MB) and computes the full
scores itself.  A collective version (partial scores + AllGather) was
measured slower: any cross-core barrier eats the per-execution device
dispatch skew (~40-50 us), which dwarfs the extra 7 MB of reads.  With
no collective, each core is fully independent.

Host-side prep (pure layout, no arithmetic): x and y are rotated so the
core's own batch shard comes first, and pre-tiled to partition-major
[128, NT, *] so every DMA is a big per-partition-contiguous transfer.

Matmul dtypes: the scores matmul and the x-transposes run as float32r
(bitcast view of the fp32 data - single-pass PE at full rate, ~tf32
precision); the output matmul runs in bf16 (xT and Weff); PSUM
accumulation is always fp32.  fp32 matmul proper would be a half-rate
LOW_HIGH double pass.

Output layout per core is [B_shard, C*O] (batch-major) so HBM writes
are fully contiguous; the host gather transposes back to [C, B, O].
"""

import ml_dtypes
import numpy as np

import concourse.bass as bass
import concourse.mybir as mybir
import concourse.tile as tile
from concourse import bacc
from concourse import bass_utils
from concourse.masks import make_identity

C, B, F, O, DK = 10, 8192, 256, 64, 5
NCORES = 8
BS = B // NCORES            # 1024 batch rows per core
NT = B // 128               # 64 batch tiles (full batch)
NTS = BS // 128             # 8 batch tiles in this core's own shard
FT = F // 128               # 2 feature tiles
CO = C * O                  # 640
NH = 2                      # output free-dim split (2 x 320 <= one PSUM bank)
CO_H = CO // NH             # 320
XCH = 8                     # batch tiles per x DMA chunk (1 MB)
F32 = mybir.dt.float32
F32R = mybir.dt.float32r
BF16 = mybir.dt.bfloat16
AF = mybir.ActivationFunctionType
AX = mybir.AxisListType

_CACHE = {}
LAST_RESULTS = None


def _emit(nc, tc, aps):
    x_d, y_d, wk_d, wq_d, wv_d, w_d, b_d, out_d = aps
    with (
        tc.tile_pool(name="consts", bufs=1) as consts,
        tc.tile_pool(name="xpool", bufs=1) as xpool,
        tc.tile_pool(name="stage", bufs=3) as stage_pool,
        tc.tile_pool(name="psum_big", bufs=2, space="PSUM") as psum_big,
        tc.tile_pool(name="psum_small", bufs=2, space="PSUM") as psum_small,
    ):
        # ==== constants first (GpSimd is otherwise idle) ================
        identity_f = consts.tile([128, 128], F32)
        make_identity(nc, identity_f)
        identity = consts.tile([128, 128], BF16)
        nc.vector.tensor_copy(identity, identity_f)
        ones_row = consts.tile([1, 128], F32)
        nc.gpsimd.memset(ones_row, 1.0)
        # E[c, c*O + o] = 1 (repeat-identity for broadcasting attn over o)
        e_rep = consts.tile([C, CO], F32)
        nc.gpsimd.memset(e_rep, 1.0)
        # keep 1 where j - O*c >= 0  (j: free index, c: partition)
        nc.gpsimd.affine_select(out=e_rep, in_=e_rep,
                                compare_op=mybir.AluOpType.is_ge,
                                fill=0.0, base=0, pattern=[[1, CO]],
                                channel_multiplier=-O)
        # keep 1 where j - O*c < O  i.e. O-1 - j + O*c >= 0
        nc.gpsimd.affine_select(out=e_rep, in_=e_rep,
                                compare_op=mybir.AluOpType.is_ge,
                                fill=0.0, base=O - 1, pattern=[[-1, CO]],
                                channel_multiplier=O)

        # ==== streaming loads (both HWDGE rings, pre-tiled layout) ======
        y_sb = xpool.tile([128, NT, C], BF16)
        nc.sync.dma_start(y_sb, y_d.rearrange("p (t c) -> p t c", c=C))
        x_sb = xpool.tile([128, NT, F], BF16)
        xr = x_d.rearrange("p (t f) -> p t f", f=F)
        bounds = list(range(0, NT - XCH, XCH)) + [NT - XCH, NT - XCH // 2, NT]
        for j, (i0, i1) in enumerate(zip(bounds[:-1], bounds[1:])):
            eng = nc.sync if j % 2 == 0 else nc.scalar
            eng.dma_start(x_sb[:, i0:i1], xr[:, i0:i1])

        # ==== x^T (bf16) for this core's own shard (first chunk) ========
        xT = [xpool.tile([128, BS], BF16, name=f"xT{h}") for h in range(FT)]
        for t in range(NTS):
            for h in range(FT):
                ps_tr = psum_small.tile([128, 128], BF16, tag="ps_tr")
                nc.tensor.transpose(ps_tr,
                                    x_sb[:, t, h * 128:(h + 1) * 128],
                                    identity)
                nc.vector.tensor_copy(xT[h][:, t * 128:(t + 1) * 128], ps_tr)

        # ==== params on the Scalar HWDGE ring ===========================
        wk_sb = consts.tile([C, DK], F32)
        wq_sb = consts.tile([C, DK], F32)
        wv_sb = consts.tile([C, 1], F32)
        bias_sb = consts.tile([1, CO], F32)
        nc.scalar.dma_start(wk_sb, wk_d)
        nc.scalar.dma_start(wq_sb, wq_d)
        nc.scalar.dma_start(wv_sb, wv_d)
        nc.scalar.dma_start(bias_sb, b_d)
        w_sb = []
        for h in range(FT):
            wt = consts.tile([128, CO], BF16, name=f"w_sb{h}")
            nc.scalar.dma_start(
                wt.rearrange("p (c o) -> p c o", c=C),
                w_d[:, h * 128:(h + 1) * 128, :].rearrange("c f o -> f c o"),
            )
            w_sb.append(wt)

        # bias broadcast [128, CO] via ones-column matmul (fills PE idle
        # before the scores stream; runs as soon as bias_sb lands)
        bias_bc = consts.tile([128, CO], F32)
        for nh in range(NH):
            sl = slice(nh * CO_H, (nh + 1) * CO_H)
            ps_b = psum_big.tile([128, CO_H], F32, tag=f"ps_o{nh}",
                                 name=f"ps_b{nh}")
            nc.tensor.matmul(ps_b, ones_row, bias_sb[:, sl],
                             start=True, stop=True)
            nc.vector.tensor_copy(bias_bc[:, sl], ps_b)

        # fold w_value into the expansion matrix: E_wv[c, c*O+o] = w_value[c]
        e_wv = consts.tile([C, CO], F32R)
        e_tmp = consts.tile([C, CO], F32)
        nc.vector.tensor_scalar_mul(e_tmp, e_rep, wv_sb)
        nc.vector.tensor_copy(e_wv, e_tmp)

        # s[c] = dot(w_key[c], w_query[c]) / B
        kq = consts.tile([C, DK], F32)
        nc.vector.tensor_mul(kq, wk_sb, wq_sb)
        skq = consts.tile([C, 1], F32)
        nc.vector.reduce_sum(skq, kq, axis=AX.X)
        nc.scalar.mul(skq, skq, 1.0 / B)

        # ==== full-batch scores S[c, f] ================================
        ps_scores = psum_small.tile([C, F], F32, bufs=1)
        for t in range(NT):
            nc.tensor.matmul(ps_scores,
                             y_sb[:, t],
                             x_sb[:, t],
                             start=(t == 0), stop=(t == NT - 1))

        # ==== attn = exp(scores - rowmax) ===============================
        scores = consts.tile([C, F], F32)
        nc.vector.tensor_scalar_mul(scores, ps_scores, skq)
        neg_max = consts.tile([C, 1], F32)
        nc.vector.reduce_max(neg_max, scores, axis=AX.X, negate=True)
        attn = consts.tile([C, F], F32R)
        nc.scalar.activation(attn, scores, AF.Exp, bias=neg_max, scale=1.0)

        # Weff[h][f, c*O+o] = w[h][f, c*O+o] * attn[c, f] * w_value[c]
        # broadcast over o via PE: attn_bc = attn.T @ E_wv, then one
        # elementwise multiply per half.
        weff = []
        for h in range(FT):
            wf = consts.tile([128, CO], BF16, name=f"weff{h}")
            lhsT = attn[:, h * 128:(h + 1) * 128]
            for nh in range(NH):
                sl = slice(nh * CO_H, (nh + 1) * CO_H)
                ps_we = psum_big.tile([128, CO_H], F32, tag=f"ps_o{nh}",
                                      name=f"ps_we{h}{nh}")
                nc.tensor.matmul(ps_we, lhsT, e_wv[:, sl],
                                 start=True, stop=True)
                nc.vector.tensor_mul(wf[:, sl], w_sb[h][:, sl], ps_we)
            weff.append(wf)

        # ==== big matmul: out[b, (c o)] = x @ Weff + bias ===============
        for t in range(NTS):
            ps_o = [psum_big.tile([128, CO_H], F32, tag=f"ps_o{nh}",
                                  name=f"ps_o{nh}_{t}")
                    for nh in range(NH)]
            for h in range(FT):
                lhsT = xT[h][:, t * 128:(t + 1) * 128]
                for nh in range(NH):
                    nc.tensor.matmul(
                        ps_o[nh], lhsT,
                        weff[h][:, nh * CO_H:(nh + 1) * CO_H],
                        start=(h == 0), stop=(h == FT - 1))
            st = stage_pool.tile([128, CO], F32, tag="st")
            for nh in range(NH):
                sl = slice(nh * CO_H, (nh + 1) * CO_H)
                nc.vector.tensor_add(st[:, sl], ps_o[nh], bias_bc[:, sl])
            eng = nc.sync if t % 2 == 0 else nc.scalar
            eng.dma_start(out_d[t * 128:(t + 1) * 128, :], st)


def _build():
    nc = bacc.Bacc("TRN2", target_bir_lowering=False, debug=False,
                   enable_asserts=False, num_devices=NCORES)
    x_d = nc.dram_tensor("x_s", [128, NT * F], BF16, kind="ExternalInput").ap()
    y_d = nc.dram_tensor("y_s", [128, NT * C], BF16, kind="ExternalInput").ap()
    wk_d = nc.dram_tensor("w_key", [C, DK], F32, kind="ExternalInput").ap()
    wq_d = nc.dram_tensor("w_query", [C, DK], F32, kind="ExternalInput").ap()
    wv_d = nc.dram_tensor("w_value", [C, 1], F32, kind="ExternalInput").ap()
    w_d = nc.dram_tensor("weight", [C, F, O], BF16, kind="ExternalInput").ap()
    b_d = nc.dram_tensor("bias", [1, CO], F32, kind="ExternalInput").ap()
    out_d = nc.dram_tensor("out", [BS, CO], F32, kind="ExternalOutput").ap()

    with tile.TileContext(nc) as tc:
        _emit(nc, tc, (x_d, y_d, wk_d, wq_d, wv_d, w_d, b_d, out_d))
    nc.compile()
    return nc


def get_nc():
    if "nc" not in _CACHE:
        _CACHE["nc"] = _build()
    return _CACHE["nc"]


def _pretile(a, r, width):
    """Rotate rows so core r's shard is first, then pre-tile to
    partition-major [128, NT*width] in bf16 (the kernel's compute
    precision for the streaming operands)."""
    k = r * BS
    rot = np.concatenate([a[k:], a[:k]], axis=0)          # [B, width]
    til = rot.reshape(NT, 128, width).transpose(1, 0, 2)  # [128, NT, width]
    return np.ascontiguousarray(
        til.reshape(128, NT * width).astype(ml_dtypes.bfloat16))


def make_in_maps(x, y, w_key, w_query, w_value, weight, bias):
    f = np.float32
    x = np.asarray(x, dtype=f)
    y = np.asarray(y, dtype=f)
    wk2 = np.ascontiguousarray(w_key.reshape(C, DK), dtype=f)
    wq2 = np.ascontiguousarray(w_query.reshape(C, DK), dtype=f)
    wv2 = np.ascontiguousarray(w_value.reshape(C, 1), dtype=f)
    w3 = np.ascontiguousarray(
        np.asarray(weight, dtype=f).reshape(C, F, O).astype(ml_dtypes.bfloat16))
    b2 = np.ascontiguousarray(bias.reshape(1, CO), dtype=f)
    in_maps = []
    for r in range(NCORES):
        in_maps.append({
            "x_s": _pretile(x, r, F),
            "y_s": _pretile(y, r, C),
            "w_key": wk2,
            "w_query": wq2,
            "w_value": wv2,
            "weight": w3,
            "bias": b2,
        })
    return in_maps


def kernel(x, y, w_key, w_query, w_value, weight, bias):
    global LAST_RESULTS
    nc = get_nc()
    in_maps = make_in_maps(x, y, w_key, w_query, w_value, weight, bias)
    res = bass_utils.run_bass_kernel_spmd(nc, in_maps,
                                          core_ids=list(range(NCORES)))
    LAST_RESULTS = res
    # each core: [BS, C*O] -> full out [C, B, O]
    shards = [np.asarray(res.results[r]["out"]).reshape(BS, C, O)
              for r in range(NCORES)]
    full = np.concatenate(shards, axis=0)          # [B, C, O]
    return np.ascontiguousarray(full.transpose(1, 0, 2))
